# revision 23
# baseline (speedup 1.0000x reference)
"""Trainium2 Bass kernel for the GRU caption model.

Computes: h0 = feat @ W_hp.T + b_hp; 200-step GRU with constant hidden-proj
gate pre-activations; logits = outs @ W_out.T (+ b_out on host) -> [B, V, T].

Sharding: hybrid 2-way batch x 4-way vocab across the 8 cores.  Core c
handles batch half c//4 (16 rows) and vocab quarter c%4 (7680 padded rows).
Each core runs its batch half's GRU; the projection uses the GRU state tiles
as the *stationary* matmul operand ([128 h, 128 (t,b)] chunks) and streams
W_out columns, so each 8-timestep "granule" yields a [128 (t,b), 7680 v]
fp16 tile that leaves in one large DMA.  PSUM->fp16 drains run on the
otherwise-idle GPSIMD engine so Act/DVE serve only the serial GRU chain.
Gate constants (W_hh @ h0 + biases) are accumulated into the gates PSUM
through a small identity matmul; the r-gate PSUM is a separate tile so the
chain's first tanh only waits on the r matmuls.
"""

import numpy as np
import ml_dtypes

import concourse.bass as bass
import concourse.mybir as mybir
import concourse.tile as tile
from concourse import bacc
from concourse.bass_utils import run_bass_kernel_spmd

F32 = mybir.dt.float32
BF16 = mybir.dt.bfloat16
FP16 = mybir.dt.float16
AF = mybir.ActivationFunctionType
ALU = mybir.AluOpType

VOCAB = 30522
HID = 512
FEAT = 2048
STEPS = 200
BATCH = 32
SOS = 101
NCORES = 8
P = 128
KO = HID // P            # 4 h-chunks
GM = 3 * HID // P        # 12 gate row-groups (r: 0-3, z: 4-7, n: 8-11)
KF = FEAT // P           # 16 feat chunks
BS = 16                  # per-core batch shard
NVQ = 4                  # vocab quarters
VPAD = 30720 // NVQ      # per-core padded vocab rows = 7680
GR = P // BS             # granule timesteps -> 128 (t,b) columns (8)
NGRAN = STEPS // GR      # 25
VC = 480                 # proj v-chunk columns (psum bank holds <=512 f32)
NVC = VPAD // VC         # 16 units per granule
UPS = NVC // GR          # proj units emitted per step (2)

LAST_RESULTS = None  # test harness introspection


def build():
    nc = bacc.Bacc("TRN2", target_bir_lowering=False, debug=False)

    featT = nc.dram_tensor("featT", [FEAT, BS], FP16, kind="ExternalInput")
    WhpT = nc.dram_tensor("WhpT", [FEAT, HID], FP16, kind="ExternalInput")
    WhhT = nc.dram_tensor("WhhT", [HID, 3 * HID], BF16, kind="ExternalInput")
    WihT = nc.dram_tensor("WihT", [HID, 3 * HID], BF16, kind="ExternalInput")
    x0T = nc.dram_tensor("x0T", [HID, BS], BF16, kind="ExternalInput")
    WoutT = nc.dram_tensor("WoutT", [HID, VPAD], BF16, kind="ExternalInput")
    b_hp = nc.dram_tensor("b_hp", [HID], F32, kind="ExternalInput")
    bsum_rz = nc.dram_tensor("bsum_rz", [2 * HID], F32, kind="ExternalInput")
    bmix_n = nc.dram_tensor("bmix_n", [HID], F32, kind="ExternalInput")
    bhhn_half = nc.dram_tensor("bhhn_half", [HID], F32, kind="ExternalInput")
    # row (t*BS + b) holds logits[b, :, t] for this core's vocab slice
    OUT = nc.dram_tensor("OUT", [STEPS * BS, VPAD], FP16, kind="ExternalOutput")

    with tile.TileContext(nc) as tc:
        with (
            tc.tile_pool(name="const", bufs=1) as const,
            tc.tile_pool(name="stage", bufs=2) as stagep,
            tc.tile_pool(name="step", bufs=3) as sp,
            tc.tile_pool(name="psr", bufs=2, space="PSUM") as psrp,
            tc.tile_pool(name="pszn", bufs=2, space="PSUM") as psznp,
            tc.tile_pool(name="psp", bufs=4, space="PSUM") as psp,
        ):
            # ---- constants into SBUF ----
            # DMA_ENGINES serialize transfers, so order by when each tensor
            # is first needed: feat/whp (h0) -> whh (G0) -> wih/x0 (step 0);
            # the big wout load is only needed once projection starts (t>=8).
            featT_sb = const.tile([P, KF, BS], FP16, tag="featsb")
            nc.sync.dma_start(featT_sb[:], featT.rearrange("(k p) b -> p k b", p=P))
            whp_sb = const.tile([P, KF, HID], FP16, tag="whp")
            whp_src = WhpT.rearrange("(k p) h -> p k h", p=P)
            nc.sync.dma_start(whp_sb[:, 0:8, :], whp_src[:, 0:8, :])
            nc.sync.dma_start(whp_sb[:, 8:16, :], whp_src[:, 8:16, :])
            whh_sb = const.tile([P, KO, 3 * HID], BF16, tag="whh")
            whh_src = WhhT.rearrange("(k p) g -> p k g", p=P)
            for gc in range(3):
                cs = slice(gc * 512, (gc + 1) * 512)
                nc.sync.dma_start(whh_sb[:, :, cs], whh_src[:, :, cs])
            wih = const.tile([P, KO, GM, P], BF16, tag="wih")
            wih_src = WihT.rearrange("(k p) (m c) -> p k m c", p=P, c=P)
            nc.sync.dma_start(wih[:, :, 0:4, :], wih_src[:, :, 0:4, :])
            nc.sync.dma_start(wih[:, :, 4:12, :], wih_src[:, :, 4:12, :])
            # small constants ride the Activation engine's DMA queue so
            # their fixed per-DMA overheads overlap SP's big weight loads
            bhp_sb = const.tile([P, KO], F32, tag="bhp")
            nc.scalar.dma_start(bhp_sb[:], b_hp.rearrange("(m p) -> p m", p=P))
            bsrz_sb = const.tile([P, 8], F32, tag="bsrz")
            nc.scalar.dma_start(bsrz_sb[:], bsum_rz.rearrange("(m p) -> p m", p=P))
            bmixn_sb = const.tile([P, KO], F32, tag="bmixn")
            nc.scalar.dma_start(bmixn_sb[:], bmix_n.rearrange("(m p) -> p m", p=P))
            bhhnh_sb = const.tile([P, KO], F32, tag="bhhnh")
            nc.scalar.dma_start(bhhnh_sb[:], bhhn_half.rearrange("(m p) -> p m", p=P))
            x0_sb = const.tile([P, KO, BS], BF16, tag="x0")
            nc.scalar.dma_start(x0_sb[:], x0T.rearrange("(k p) b -> p k b", p=P))
            # wout in v-chunks: projection unit u only needs its own chunk,
            # so granule 0 can start before the whole 7.9MB lands
            wout = const.tile([P, KO, VPAD], BF16, tag="wout")
            wout_src = WoutT.rearrange("(k p) v -> p k v", p=P)
            WCH = VPAD // 4
            for wc in range(4):
                ws = slice(wc * WCH, (wc + 1) * WCH)
                nc.sync.dma_start(wout[:, :, ws], wout_src[:, :, ws])

            # resT[p, k, t, b] = h_{t+1}[k*128+p, b]; (t, b) last so an
            # 8-step granule slice is a contiguous 128-column stationary
            # operand for the projection matmuls.
            resT = const.tile([P, KO, STEPS, BS], BF16, tag="resT")

            # ---- h0 = feat @ W_hp.T + b_hp ----
            # two accumulation passes into the same psum slices so the second
            # whp DMA half overlaps the first half's matmuls
            ps_h = psznp.tile([P, 8, BS], F32, tag="gzn")
            for half in range(2):
                for ko in range(KO):
                    for kf in range(8 * half, 8 * half + 8):
                        nc.tensor.matmul(
                            ps_h[:, ko, :],
                            whp_sb[:, kf, ko * P:(ko + 1) * P],
                            featT_sb[:, kf, :],
                            start=(kf == 0), stop=(kf == KF - 1),
                            skip_group_check=True,
                        )
            h0T = const.tile([P, KO, BS], F32, tag="h0T")
            for ko in range(KO):
                nc.scalar.activation(
                    h0T[:, ko, :], ps_h[:, ko, :], AF.Identity,
                    bias=bhp_sb[:, ko, None], scale=1.0,
                )
            h0h = const.tile([P, KO, BS], BF16, tag="h0h")
            nc.scalar.mul(h0h[:], h0T[:], 0.5)
            h0b = const.tile([P, KO, BS], BF16, tag="h0b")
            nc.scalar.copy(h0b[:], h0T[:])

            # ---- gate constants in the gates layout (bf16 SBUF) ----
            # rz rows: g0 = W_hh@h0 + b_hh + b_ih
            # n rows:  g0 = 0.5*(W_hh@h0 + b_hh) + b_ih   (E_n form)
            # Each step the Pool engine preloads these into the gates PSUM,
            # replacing twelve per-step PE matmuls.
            ps_rz = psznp.tile([P, 8, BS], F32, tag="gzn")
            for m in range(8):
                for k in range(KO):
                    nc.tensor.matmul(
                        ps_rz[:, m, :],
                        whh_sb[:, k, m * P:(m + 1) * P],
                        h0b[:, k, :],
                        start=(k == 0), stop=(k == KO - 1),
                    )
            ps_n = psrp.tile([P, 4, BS], F32, tag="gr")
            for m in range(4):
                for k in range(KO):
                    nc.tensor.matmul(
                        ps_n[:, m, :],
                        whh_sb[:, k, (m + 8) * P:(m + 9) * P],
                        h0b[:, k, :],
                        start=(k == 0), stop=(k == KO - 1),
                    )
            g0_sb = const.tile([P, GM, BS], BF16, tag="g0sb")
            nc.vector.tensor_add(
                g0_sb[:, 0:4, :], ps_rz[:, 0:4, :],
                bsrz_sb[:, 0:4, None].to_broadcast((P, 4, BS)),
            )
            nc.vector.tensor_add(
                g0_sb[:, 4:8, :], ps_rz[:, 4:8, :],
                bsrz_sb[:, 4:8, None].to_broadcast((P, 4, BS)),
            )
            nc.vector.scalar_tensor_tensor(
                g0_sb[:, 8:12, :], ps_n[:], 0.5,
                bmixn_sb[:, :, None].to_broadcast((P, KO, BS)),
                ALU.mult, ALU.add,
            )
            hn2 = const.tile([P, KO, BS], BF16, tag="hn2")
            nc.vector.scalar_tensor_tensor(
                hn2[:], ps_n[:], 0.5,
                bhhnh_sb[:, :, None].to_broadcast((P, KO, BS)),
                ALU.mult, ALU.add,
            )

            # ---- per-granule projection state ----
            stage_tiles = {}

            def emit_proj_mm(g, u):
                if u == 0:
                    stage_tiles[g] = stagep.tile(
                        [P, VPAD], FP16, tag="stage", name=f"stage{g}"
                    )
                pp = psp.tile([P, VC], F32, tag="pp", name=f"pp{g}_{u}")
                for k in range(KO):
                    nc.tensor.matmul(
                        pp[:],
                        resT[:, k, g * GR:(g + 1) * GR, :],
                        wout[:, k, u * VC:(u + 1) * VC],
                        start=(k == 0), stop=(k == KO - 1),
                    )
                return pp

            def emit_proj_tail(g, u, pp):
                # GPSIMD cannot touch PSUM on real TRN2, so the PSUM->fp16
                # drains alternate between Act and DVE.
                st = stage_tiles[g]
                if u % 2 == 0:
                    nc.scalar.copy(st[:, u * VC:(u + 1) * VC], pp[:])
                else:
                    nc.vector.tensor_scalar_add(st[:, u * VC:(u + 1) * VC], pp[:], 0.0)
                # two half DMAs per granule so the first half's store overlaps
                # the second half's matmuls/converts
                half = NVC // 2 * VC
                if u == NVC // 2 - 1:
                    nc.sync.dma_start(
                        OUT[g * P:(g + 1) * P, 0:half], st[:, 0:half]
                    )
                elif u == NVC - 1:
                    nc.sync.dma_start(
                        OUT[g * P:(g + 1) * P, half:VPAD], st[:, half:VPAD]
                    )
                    del stage_tiles[g]

            # ---- GRU steps ----
            # The gate psum tiles are preloaded with the constant terms one
            # step ahead (no h dependency) so the accumulating matmuls never
            # wait on the preload.
            def alloc_gates(i):
                psr = psrp.tile([P, 4, BS], F32, tag="gr", name=f"gr{i}")
                pszn = psznp.tile([P, 8, BS], F32, tag="gzn", name=f"gzn{i}")
                nc.scalar.copy(psr[:], g0_sb[:, 0:4, :])
                nc.vector.tensor_scalar_add(pszn[:], g0_sb[:, 4:12, :], 0.0)
                return psr, pszn

            nxt_gates = alloc_gates(0)
            for t in range(STEPS):
                psr, pszn = nxt_gates
                for m in range(GM):
                    dst = psr[:, m, :] if m < 4 else pszn[:, m - 4, :]
                    for k in range(KO):
                        rhs = x0_sb[:, k, :] if t == 0 else resT[:, k, t - 1, :]
                        nc.tensor.matmul(
                            dst, wih[:, k, m, :], rhs,
                            start=False, stop=(k == KO - 1),
                            skip_group_check=True,
                        )
                # projection units (granule g = t//GR - 1) interleave here to
                # fill the PE stream while the elementwise chain runs
                if t + 1 < STEPS:
                    nxt_gates = alloc_gates(t + 1)
                pps = []
                g = t // GR - 1
                if g >= 0:
                    for u in range(UPS * (t % GR), UPS * (t % GR) + UPS):
                        pps.append((g, u, emit_proj_mm(g, u)))

                tr = sp.tile([P, KO, BS], BF16, tag="tr")
                nc.scalar.activation(tr[:], psr[:], AF.Tanh, scale=0.5)
                tz = sp.tile([P, KO, BS], BF16, tag="tz")
                nc.scalar.activation(tz[:], pszn[:, 0:4, :], AF.Tanh, scale=0.5)
                a = sp.tile([P, KO, BS], BF16, tag="a")
                nc.vector.tensor_mul(a[:], tr[:], hn2[:])
                sn = sp.tile([P, KO, BS], BF16, tag="sn")
                nc.vector.tensor_add(sn[:], pszn[:, 4:8, :], a[:])
                n_ = sp.tile([P, KO, BS], BF16, tag="n")
                nc.scalar.activation(n_[:], sn[:], AF.Tanh, scale=1.0)
                # d = 0.5 - 0.5*tz ; c1 = h0h*(1+tz) = h0h + h0h*tz
                d = sp.tile([P, KO, BS], BF16, tag="d")
                nc.gpsimd.tensor_scalar(d[:], tz[:], -0.5, 0.5, ALU.mult, ALU.add)
                u_ = sp.tile([P, KO, BS], BF16, tag="u")
                nc.gpsimd.tensor_mul(u_[:], tz[:], h0h[:])
                c1 = sp.tile([P, KO, BS], BF16, tag="c1")
                nc.gpsimd.tensor_add(c1[:], u_[:], h0h[:])
                e = sp.tile([P, KO, BS], BF16, tag="e")
                nc.vector.tensor_mul(e[:], n_[:], d[:])
                # h' = e + c1, written straight into the res history
                nc.vector.tensor_add(resT[:, :, t, :], e[:], c1[:])

                for g, u, pp in pps:
                    emit_proj_tail(g, u, pp)

            # ---- drain the last granule's projection ----
            g = NGRAN - 1
            for u in range(NVC):
                pp = emit_proj_mm(g, u)
                emit_proj_tail(g, u, pp)

    nc.compile()
    return nc


def _shard_inputs(feat, W_hp, b_hp, W_ih, W_hh, b_ih, b_hh, embed, W_out, b_out):
    bf = ml_dtypes.bfloat16
    feat = np.asarray(feat)
    WhpT = np.ascontiguousarray(np.asarray(W_hp).T, dtype=np.float16)
    WhhT = np.ascontiguousarray(np.asarray(W_hh).T).astype(bf)
    WihT = np.ascontiguousarray(np.asarray(W_ih).T).astype(bf)
    x0T = np.ascontiguousarray(
        np.repeat(np.asarray(embed)[SOS][:, None], BS, axis=1)
    ).astype(bf)
    b_ih = np.asarray(b_ih, np.float32)
    b_hh = np.asarray(b_hh, np.float32)
    bsum_rz = (b_hh + b_ih)[:2 * HID].copy()
    bmix_n = (0.5 * b_hh + b_ih)[2 * HID:].copy()
    bhhn_half = (0.5 * b_hh)[2 * HID:].copy()
    Wo = np.zeros((NVQ * VPAD, HID), np.float32)
    Wo[:VOCAB] = np.asarray(W_out)
    common = dict(
        WhpT=WhpT, WhhT=WhhT, WihT=WihT, x0T=x0T,
        b_hp=np.asarray(b_hp, np.float32),
        bsum_rz=bsum_rz, bmix_n=bmix_n, bhhn_half=bhhn_half,
    )
    featT_halves = [
        np.ascontiguousarray(feat[hb * BS:(hb + 1) * BS].T, dtype=np.float32)
        for hb in range(2)
    ]
    woutT_quarters = [
        np.ascontiguousarray(Wo[vq * VPAD:(vq + 1) * VPAD].T).astype(bf)
        for vq in range(NVQ)
    ]
    in_maps = []
    for c in range(NCORES):
        hb, vq = divmod(c, NVQ)
        m = dict(common)
        m["featT"] = featT_halves[hb]
        m["WoutT"] = woutT_quarters[vq]
        in_maps.append(m)
    return in_maps


def kernel(**inputs):
    global LAST_RESULTS
    args = {k: np.asarray(v) for k, v in inputs.items()}
    in_maps = _shard_inputs(
        args["feat"], args["W_hp"], args["b_hp"], args["W_ih"], args["W_hh"],
        args["b_ih"], args["b_hh"], args["embed"], args["W_out"], args["b_out"],
    )
    nc = build()
    res = run_bass_kernel_spmd(nc, in_maps, core_ids=list(range(NCORES)))
    LAST_RESULTS = res
    full = np.empty((BATCH, VOCAB, STEPS), np.float32)
    for c in range(NCORES):
        hb, vq = divmod(c, NVQ)
        v0 = vq * VPAD
        nv = min(VPAD, VOCAB - v0)
        if nv <= 0:
            continue
        # OUT is [(T*BS), VPAD] fp16, row t*BS + b
        o = np.asarray(res.results[c]["OUT"], dtype=np.float32)
        o = o.reshape(STEPS, BS, VPAD)
        full[hb * BS:(hb + 1) * BS, v0:v0 + nv, :] = (
            o[:, :, :nv].transpose(1, 2, 0)
        )
    b_out = np.asarray(args["b_out"], np.float32)
    if np.any(b_out):
        full += b_out[None, :, None]
    return np.ascontiguousarray(full, dtype=np.float32)
